# revision 25
# baseline (speedup 1.0000x reference)
"""BotRGCN (4 shared RGCN layers) on 8 TRN2 NeuronCores via Bass/Tile.

Strategy (sharding_hint): nodes sharded across 8 cores (6250 each, padded to
6656 = 13*512); edges partitioned by destination core and sorted by
(dst_chunk, col) where col = (node%512)*R + rel; per layer an AllGather
replicates the row-major x table (fp16, two source-half streams so gather
row ids fit int16) to every core's DRAM; each core dma_gathers its edges'
source rows (4 SWDGE queues), scales each gathered row by 1/cnt(segment) on
the vector engine, and segment-sums via PE matmuls against a RESIDENT fp8
0/1 membership matrix. Spans are chunk-level (cap 128 edges across cores,
not bank aligned) for ~86% slot fill; per-chunk matmuls are emitted
round-robin across the 5 PSUM banks. Table halves for layer l+1 are
transposed + AllGathered mid-layer-l (after chunks 6/12) to hide collective
latency. Weights replicated; all shapes baked at compile time.

Self-contained: hardcodes all shapes from the problem spec.
"""
import os
import time

import numpy as np

import concourse.bacc as bacc
import concourse.bass as bass
import concourse.mybir as mybir
import concourse.tile as tile
from concourse.bass_utils import run_bass_kernel_spmd
from concourse.masks import make_identity

# ---------------- problem constants (hardcoded from spec) ----------------
NCORES = 8
N = 50000
E = 800000
R = 5
D = 128
FIN = 768 + 768 + 6 + 11          # 1553 concat input features
FINP = 13 * 128                   # padded to 1664
NLOC = N // NCORES                # 6250
CHUNK = 512                       # nodes per chunk
NCHUNK = 13
NPAD = NCHUNK * CHUNK             # 6656 padded nodes/core
CCOLS = CHUNK * R                 # 2560 segment cols per chunk
NSEG = NPAD * R                   # 33280 dense segment grid per core
BANK = 512                        # psum bank cols (fp32)
BANKS_PER_CHUNK = CCOLS // BANK   # 5
HALFROW = NPAD // 2               # 3328 rows per half-table shard
NTABH = NCORES * HALFROW          # 26624 rows per half table (< 32768)
SLOTS = 128                       # edge slots per tile
SUBT = 8                          # tiles per dma_gather call
NLAYER = int(os.environ.get("KB_LAYERS", "4"))
SKIP = set(os.environ.get("KB_SKIP", "").split(","))

F16 = mybir.dt.float16
F32 = mybir.dt.float32
F8 = mybir.dt.float8e4
I16 = mybir.dt.int16

_CACHE = {}


# ---------------- host-side graph preprocessing ----------------
def _plan_graph(edge_index, edge_type):
    """Chunk-level spans shared across cores (SPMD); per-core gather idx,
    fp8 0/1 membership matrix, per-slot 1/cnt scale vectors."""
    src = np.asarray(edge_index[0], dtype=np.int64)
    dst = np.asarray(edge_index[1], dtype=np.int64)
    et = np.asarray(edge_type, dtype=np.int64)

    core = dst // NLOC
    node_loc = dst % NLOC
    chunk = node_loc // CHUNK
    colc = (node_loc % CHUNK) * R + et                 # 0..CCOLS-1
    src_core = src // NLOC
    src_loc = src % NLOC
    stream = (src_loc >= HALFROW).astype(np.int64)
    hl = src_loc - stream * HALFROW                    # 0..HALFROW-1
    # wrapped table layout: shard written [128 part, 26 blk, 128 d]
    src_row = src_core * HALFROW + (hl % 128) * 26 + (hl // 128)

    counts = np.zeros((NCORES, 2, NCHUNK, CCOLS), np.int64)
    np.add.at(counts, (core, stream, chunk, colc), 1)
    cnt_seg = counts.sum(axis=1)                       # [NCORES,NCHUNK,CCOLS]
    invc = 1.0 / np.maximum(cnt_seg, 1.0)

    # spans per (stream, chunk): contiguous cover of [0, CCOLS), greedy so
    # max-over-cores count per span <= SLOTS; spans may cross bank bounds
    spans = {0: [], 1: []}
    for s in range(2):
        for c in range(NCHUNK):
            cc = counts[:, s, c, :]
            assert cc.max(initial=0) <= SLOTS
            widths = []
            run = np.zeros(NCORES, np.int64)
            w = 0
            for j in range(CCOLS):
                if (run + cc[:, j]).max() > SLOTS:
                    widths.append(w)
                    run[:] = 0
                    w = 0
                run += cc[:, j]
                w += 1
            widths.append(w)
            spans[s].append(widths)
    ntiles = {s: [len(spans[s][c]) for c in range(NCHUNK)] for s in range(2)}
    tot_tiles = {s: sum(ntiles[s]) for s in range(2)}
    ttot = tot_tiles[0] + tot_tiles[1]
    svoff = {0: 0, 1: tot_tiles[0]}

    # per-bank matmul op lists per chunk: (s, tloc_in_chunk, lo, hi) in
    # chunk-col coords; tiles split at bank boundaries
    ops = []
    for c in range(NCHUNK):
        bops = [[] for _ in range(BANKS_PER_CHUNK)]
        for s in range(2):
            lo = 0
            for t, w in enumerate(spans[s][c]):
                hi = lo + w
                for b in range(lo // BANK, (hi - 1) // BANK + 1):
                    pl = max(lo, b * BANK)
                    ph = min(hi, (b + 1) * BANK)
                    bops[b].append((s, t, pl, ph))
                lo = hi
            assert lo == CCOLS
        ops.append(bops)

    # per-core data: gather idx + fp16 M with 1/cnt baked in
    gidx = {}
    m01 = np.zeros((NCORES, 128, 2 * NCHUNK * CCOLS), np.float16)
    order = np.lexsort((colc, chunk, stream, core))    # sort edges
    o_core, o_str = core[order], stream[order]
    o_chunk, o_col = chunk[order], colc[order]
    o_srcrow = src_row[order]
    for k in range(NCORES):
        for s in range(2):
            flat = np.zeros((tot_tiles[s], SLOTS), np.int16)
            toff = 0
            for c in range(NCHUNK):
                m = (o_core == k) & (o_str == s) & (o_chunk == c)
                cols_e = o_col[m]
                rows_e = o_srcrow[m]
                T = ntiles[s][c]
                bounds = np.cumsum([0] + spans[s][c])
                e_b = np.searchsorted(cols_e, bounds)
                nsl = e_b[1:] - e_b[:-1]
                assert nsl.max(initial=0) <= SLOTS
                span_of = np.repeat(np.arange(T), nsl)
                slot = np.arange(len(cols_e)) - e_b[span_of]
                flat[toff + span_of, slot] = rows_e
                m01[k, slot, (s * NCHUNK + c) * CCOLS + cols_e] = \
                    invc[k, c, cols_e].astype(np.float16)
                toff += T
            wr = flat.reshape(-1, 16).T                # [16, tot*8]
            gidx[(k, s)] = np.tile(wr, (8, 1))         # [128, tot*8]
    return dict(spans=spans, ntiles=ntiles, tot_tiles=tot_tiles,
                svoff=svoff, ops=ops, gidx=gidx, m01=m01, ttot=ttot)


# ---------------- device program ----------------
def _build_nc(plan):
    nc = bacc.Bacc("TRN2", target_bir_lowering=False, debug=False,
                   num_devices=NCORES, num_swdge_queues=4,
                   dynamic_dma_scratch_size=32768)
    spans, ntiles = plan["spans"], plan["ntiles"]
    tot_tiles, svoff = plan["tot_tiles"], plan["svoff"]
    ops, ttot = plan["ops"], plan["ttot"]

    featT = nc.dram_tensor("featT", [FINP, NPAD], F16, kind="ExternalInput")
    w_all = nc.dram_tensor("w_all", [128, 13 * 128], F16, kind="ExternalInput")
    b_x0 = nc.dram_tensor("b_x0", [128, 1], F32, kind="ExternalInput")
    w_in = nc.dram_tensor("w_in", [128, 128], F16, kind="ExternalInput")
    b_in = nc.dram_tensor("b_in", [128, 1], F32, kind="ExternalInput")
    relw = nc.dram_tensor("relw", [128, R * 128], F16, kind="ExternalInput")
    rootw = nc.dram_tensor("rootw", [128, 128], F16, kind="ExternalInput")
    rgcn_b = nc.dram_tensor("rgcn_b", [128, 1], F32, kind="ExternalInput")
    wo1 = nc.dram_tensor("wo1", [128, 128], F16, kind="ExternalInput")
    b_o1 = nc.dram_tensor("b_o1", [128, 1], F32, kind="ExternalInput")
    wo2 = nc.dram_tensor("wo2", [128, 2], F16, kind="ExternalInput")
    b_o2 = nc.dram_tensor("b_o2", [2, 1], F32, kind="ExternalInput")
    gidxA = nc.dram_tensor("gidxA", [128, tot_tiles[0] * 8], I16,
                           kind="ExternalInput")
    gidxB = nc.dram_tensor("gidxB", [128, tot_tiles[1] * 8], I16,
                           kind="ExternalInput")
    m01d = nc.dram_tensor("m01d", [128, 2 * NCHUNK * CCOLS], F16,
                          kind="ExternalInput")
    outT = nc.dram_tensor("outT", [2, NPAD], F32, kind="ExternalOutput")

    with tile.TileContext(nc) as tc:
        with (
            tc.tile_pool(name="const", bufs=1) as constp,
            tc.tile_pool(name="xt", bufs=2) as xtp,
            tc.tile_pool(name="stage", bufs=1) as stagep,
            tc.tile_pool(name="feat", bufs=3) as featp,
            tc.tile_pool(name="gb", bufs=6) as gbp,
            tc.tile_pool(name="msb", bufs=2) as msbp,
            tc.tile_pool(name="stile", bufs=2) as stp,
            tc.tile_pool(name="small", bufs=3) as smallp,
            tc.tile_pool(name="pbank", bufs=1, space="PSUM") as pbank,
            tc.tile_pool(name="pbig", bufs=2, space="PSUM") as pbig,
            tc.tile_pool(name="ptp", bufs=1, space="PSUM") as ptpp,
            tc.tile_pool(name="dram", bufs=2, space="DRAM") as dramp,
            tc.tile_pool(name="shared", bufs=1, space="DRAM") as sharedp,
        ):
            # ---- resident constants ----
            def load_const(t, shape, dt):
                s = constp.tile(shape, dt, tag=t.name, name=t.name + "_s")
                nc.sync.dma_start(s[:], t[:])
                return s
            w_all_s = load_const(w_all, [128, 13 * 128], F16)
            b_x0_s = load_const(b_x0, [128, 1], F32)
            w_in_s = load_const(w_in, [128, 128], F16)
            b_in_s = load_const(b_in, [128, 1], F32)
            relw_s = load_const(relw, [128, R * 128], F16)
            rootw_s = load_const(rootw, [128, 128], F16)
            rgcn_b_s = load_const(rgcn_b, [128, 1], F32)
            wo1_s = load_const(wo1, [128, 128], F16)
            b_o1_s = load_const(b_o1, [128, 1], F32)
            wo2_s = load_const(wo2, [128, 2], F16)
            b_o2_s = load_const(b_o2, [2, 1], F32)
            gidx_s = [load_const(gidxA, [128, tot_tiles[0] * 8], I16),
                      load_const(gidxB, [128, tot_tiles[1] * 8], I16)]
            ident = constp.tile([128, 128], F16, tag="ident")
            make_identity(nc, ident[:])

            tabs = {}

            def kick_half(xsrc, l, h):
                """Transpose half h of xsrc into wrapped row-major, DMA to
                DRAM (contiguous), AllGather into this layer's table."""
                tstage = stagep.tile([128, HALFROW], F16, tag=f"tstage{h}",
                                     name=f"tstage{h}")
                for j in range(NPAD // 256):
                    jj = h * (NPAD // 256) + j
                    pt = ptpp.tile([128, 128], F16, space="PSUM", tag="ptp",
                                   name="pt")
                    nc.tensor.transpose(pt[:],
                                        xsrc[:, jj * 128:(jj + 1) * 128],
                                        ident[:])
                    nc.scalar.activation(
                        tstage[:, j * 128:(j + 1) * 128], pt[:],
                        mybir.ActivationFunctionType.Identity)
                tsh = dramp.tile([128, HALFROW], F16, tag=f"tsh{h}",
                                 name=f"tsh{h}")
                nc.sync.dma_start(tsh[:], tstage[:])
                tb = sharedp.tile([NTABH, D], F16, addr_space="Shared",
                                  tag=f"tab{l}_{h}", name=f"tab{l}_{h}")
                if "coll" not in SKIP:
                    nc.gpsimd.collective_compute(
                        "AllGather", mybir.AluOpType.bypass,
                        replica_groups=[list(range(NCORES))],
                        ins=[tsh[:].opt()], outs=[tb[:].opt()])
                else:
                    nc.sync.dma_start(
                        tb[NTABH // 2:NTABH // 2 + HALFROW, :].rearrange(
                            "(p j) d -> p (j d)", p=128), tsh[:])
                tabs[(l, h)] = tb

            # ---- input projection -> xT [128, NPAD] fp16 ----
            xT = xtp.tile([128, NPAD], F16, tag="xT")
            for c in range(NCHUNK):
                cs = slice(c * CHUNK, (c + 1) * CHUNK)
                p0 = pbig.tile([128, CHUNK], F32, space="PSUM", tag="pbig")
                for f in range(13):
                    ft = featp.tile([128, CHUNK], F16, tag="feat")
                    if f % 2 == 0:
                        nc.sync.dma_start(ft[:],
                                          featT[f * 128:(f + 1) * 128, cs])
                    else:
                        nc.scalar.dma_start(ft[:],
                                            featT[f * 128:(f + 1) * 128, cs])
                    nc.tensor.matmul(p0[:],
                                     lhsT=w_all_s[:, f * 128:(f + 1) * 128],
                                     rhs=ft[:], start=(f == 0), stop=(f == 12))
                x0 = smallp.tile([128, CHUNK], F16, tag="x0")
                nc.scalar.activation(x0[:], p0[:],
                                     mybir.ActivationFunctionType.Lrelu,
                                     bias=b_x0_s[:], scale=1.0, alpha=0.01)
                p1 = pbig.tile([128, CHUNK], F32, space="PSUM", tag="pbig")
                nc.tensor.matmul(p1[:], lhsT=w_in_s[:], rhs=x0[:],
                                 start=True, stop=True)
                nc.scalar.activation(xT[:, cs], p1[:],
                                     mybir.ActivationFunctionType.Lrelu,
                                     bias=b_in_s[:], scale=1.0, alpha=0.01)
                if c == 6:
                    kick_half(xT, 0, 0)
                elif c == 12:
                    kick_half(xT, 0, 1)

            # ---- RGCN layers ----
            for layer in range(NLAYER):
                xTn = xtp.tile([128, NPAD], F16, tag="xT", name="xTn")
                goffs = {0: 0, 1: 0}
                toffs = {0: 0, 1: 0}
                gcall = {0: 0, 1: 0}
                for c in range(NCHUNK):
                    gtiles = {}
                    for s in range(2):
                        T = ntiles[s][c]
                        view = tabs[(layer, s)][:]
                        subs = []
                        for t0 in range(0, T, SUBT):
                            nt = min(SUBT, T - t0)
                            gb = gbp.tile([128, SUBT, D], F16, tag=f"gb{s}")
                            ni = nt * SLOTS
                            if "gather" not in SKIP:
                                nc.gpsimd.dma_gather(
                                    gb[:, :nt, :], view, gidx_s[s][
                                        :, goffs[s]:goffs[s] + ni // 16],
                                    ni, ni, D,
                                    queue_num=2 * s + (gcall[s] % 2))
                                gcall[s] += 1
                            else:
                                nc.vector.memset(gb[:, :nt, :], 0.0)
                            goffs[s] += ni // 16
                            subs.append(gb)
                        gtiles[s] = subs
                    # stream this chunk's M blocks (both streams) via HWDGE
                    msc = {}
                    for s in range(2):
                        m = msbp.tile([128, CCOLS], F16, tag=f"msb{s}")
                        mc = (s * NCHUNK + c) * CCOLS
                        if (c + s) % 2 == 0:
                            nc.sync.dma_start(m[:], m01d[:, mc:mc + CCOLS])
                        else:
                            nc.scalar.dma_start(m[:], m01d[:, mc:mc + CCOLS])
                        msc[s] = m
                    # segment-sum matmuls, round-robin across the 5 banks
                    st = stp.tile([128, CCOLS], F16, tag="stile")
                    pbs = [pbank.tile([128, BANK], F32, space="PSUM",
                                      tag=f"pbank{b}", name=f"pb{b}")
                           for b in range(BANKS_PER_CHUNK)]
                    bops = ops[c]
                    for i in range(max(len(x) for x in bops)):
                        for b in range(BANKS_PER_CHUNK):
                            if i >= len(bops[b]):
                                continue
                            s, t, pl, ph = bops[b][i]
                            nc.tensor.matmul(
                                pbs[b][:, pl - b * BANK:ph - b * BANK],
                                lhsT=gtiles[s][t // SUBT][:, t % SUBT, :],
                                rhs=msc[s][:, pl:ph],
                                start=(i == 0),
                                stop=(i == len(bops[b]) - 1))
                            if i == len(bops[b]) - 1:
                                nc.scalar.activation(
                                    st[:, b * BANK:(b + 1) * BANK], pbs[b][:],
                                    mybir.ActivationFunctionType.Identity)
                    toffs[0] += ntiles[0][c]
                    toffs[1] += ntiles[1][c]
                    # per-relation + root matmuls
                    cs = slice(c * CHUNK, (c + 1) * CHUNK)
                    po = pbig.tile([128, CHUNK], F32, space="PSUM", tag="pbig")
                    str_ap = st[:].rearrange("p (n r) -> p r n", r=R)
                    for r in range(R):
                        nc.tensor.matmul(po[:],
                                         lhsT=relw_s[:, r * 128:(r + 1) * 128],
                                         rhs=str_ap[:, r, :],
                                         start=(r == 0), stop=False)
                    nc.tensor.matmul(po[:], lhsT=rootw_s[:], rhs=xT[:, cs],
                                     start=False, stop=True)
                    nc.scalar.activation(xTn[:, cs], po[:],
                                         mybir.ActivationFunctionType.Identity,
                                         bias=rgcn_b_s[:], scale=1.0)
                    if layer < NLAYER - 1:
                        if c == 6:
                            kick_half(xTn, layer + 1, 0)
                        elif c == 12:
                            kick_half(xTn, layer + 1, 1)
                xT = xTn

            # ---- output head ----
            for c in range(NCHUNK):
                cs = slice(c * CHUNK, (c + 1) * CHUNK)
                p1 = pbig.tile([128, CHUNK], F32, space="PSUM", tag="pbig")
                nc.tensor.matmul(p1[:], lhsT=wo1_s[:], rhs=xT[:, cs],
                                 start=True, stop=True)
                h = smallp.tile([128, CHUNK], F16, tag="x0")
                nc.scalar.activation(h[:], p1[:],
                                     mybir.ActivationFunctionType.Lrelu,
                                     bias=b_o1_s[:], scale=1.0, alpha=0.01)
                p2 = pbig.tile([128, CHUNK], F32, space="PSUM", tag="pbig")
                nc.tensor.matmul(p2[:2, :], lhsT=wo2_s[:], rhs=h[:],
                                 start=True, stop=True)
                ot = smallp.tile([2, CHUNK], F32, tag="ot")
                nc.scalar.activation(ot[:], p2[:2, :],
                                     mybir.ActivationFunctionType.Identity,
                                     bias=b_o2_s[:], scale=1.0)
                nc.sync.dma_start(outT[:, cs], ot[:])

    nc.compile()
    return nc


# ---------------- host wrapper ----------------
def _pack_inputs(inputs, plan):
    f16 = np.float16
    des, tweet = inputs["des"], inputs["tweet"]
    num_prop, cat_prop = inputs["num_prop"], inputs["cat_prop"]

    w_blk = np.zeros((FINP, 128), np.float32)
    w_blk[0:768, 0:32] = inputs["W_des"]
    w_blk[768:1536, 32:64] = inputs["W_tw"]
    w_blk[1536:1542, 64:96] = inputs["W_np"]
    w_blk[1542:1553, 96:128] = inputs["W_cp"]
    w_all = np.concatenate([w_blk[f * 128:(f + 1) * 128, :]
                            for f in range(13)], axis=1).astype(f16)
    b_x0 = np.concatenate([inputs["b_des"], inputs["b_tw"],
                           inputs["b_np"], inputs["b_cp"]]
                          ).astype(np.float32).reshape(128, 1)
    relw = np.concatenate([inputs["rel_w"][r] for r in range(R)],
                          axis=1).astype(f16)

    in_maps = []
    for k in range(NCORES):
        rows = slice(k * NLOC, (k + 1) * NLOC)
        feat = np.zeros((FINP, NPAD), f16)
        feat[0:768, :NLOC] = des[rows].T
        feat[768:1536, :NLOC] = tweet[rows].T
        feat[1536:1542, :NLOC] = num_prop[rows].T
        feat[1542:1553, :NLOC] = cat_prop[rows].T
        m = {
            "featT": feat,
            "w_all": w_all,
            "b_x0": b_x0,
            "w_in": inputs["W_in"].astype(f16),
            "b_in": inputs["b_in"].astype(np.float32).reshape(128, 1),
            "relw": relw,
            "rootw": inputs["root_w"].astype(f16),
            "rgcn_b": inputs["rgcn_b"].astype(np.float32).reshape(128, 1),
            "wo1": inputs["W_o1"].astype(f16),
            "b_o1": inputs["b_o1"].astype(np.float32).reshape(128, 1),
            "wo2": inputs["W_o2"].astype(f16),
            "b_o2": inputs["b_o2"].astype(np.float32).reshape(2, 1),
            "gidxA": plan["gidx"][(k, 0)],
            "gidxB": plan["gidx"][(k, 1)],
            "m01d": plan["m01"][k],
        }
        in_maps.append(m)
    return in_maps


def _get_compiled(edge_index, edge_type):
    key = hash((np.asarray(edge_index).tobytes(),
                np.asarray(edge_type).tobytes()))
    if key not in _CACHE:
        t0 = time.time()
        plan = _plan_graph(edge_index, edge_type)
        t1 = time.time()
        nc = _build_nc(plan)
        t2 = time.time()
        print(f"[kernel] plan {t1-t0:.0f}s, build+compile {t2-t1:.0f}s",
              flush=True)
        _CACHE[key] = (nc, plan)
    return _CACHE[key]


def kernel(trace=False, **inputs):
    nc, plan = _get_compiled(inputs["edge_index"], inputs["edge_type"])
    in_maps = _pack_inputs(inputs, plan)
    t0 = time.time()
    res = run_bass_kernel_spmd(nc, in_maps, list(range(NCORES)), trace=trace)
    print(f"[kernel] run {time.time()-t0:.0f}s", flush=True)
    out = np.zeros((N, 2), np.float32)
    for k in range(NCORES):
        out[k * NLOC:(k + 1) * NLOC] = res.results[k]["outT"][:, :NLOC].T
    if trace:
        return out, res
    return out
